# revision 1
# baseline (speedup 1.0000x reference)
"""Bi-LSTM (B=64, T=512, D=H=512, no bias) on 8 Trainium2 NeuronCores.

Sharding: cores 0-3 run the forward direction on batch slices of 16,
cores 4-7 run the backward direction on the same slices (time-reversed
input / output handled on host). All 8 cores run an identical SPMD
program: a windowed input projection (bulk matmuls, 32 steps at a time)
feeding a sequential LSTM recurrence held entirely on-chip.

Per-core device layout:
  - Gate rows are permuted so m-tile m = (c, g): c = h-chunk (128 rows),
    g = gate (i, f, g, o). Permuted row = (c*4+g)*128 + r.
  - gates PSUM tile per step: [128, 256], col = c*64 + g*16 + b.
  - h (bf16) / c (fp32) state: [128, 64], col = c*16 + b, h-index = c*128+p.
  - The input projection for step s is accumulated into the gates PSUM
    via an identity matmul; recurrent matmuls (Whh^T stationary) add the
    h contribution; ScalarE applies sigmoid/tanh straight from PSUM.
All matmul operands are bf16 (fp32 PSUM accumulation); c is carried fp32.
"""

import os
import sys

for _p in ("/opt/trn_rl_repo", "/root/.axon_site/_ro/trn_rl_repo"):
    if os.path.isdir(_p) and _p not in sys.path:
        sys.path.insert(0, _p)

import numpy as np
import ml_dtypes

import concourse.mybir as mybir
import concourse.tile as tile
from concourse import bacc
from concourse.bass import ds
from concourse.bass_utils import run_bass_kernel_spmd

F32 = mybir.dt.float32
BF16 = mybir.dt.bfloat16
AF = mybir.ActivationFunctionType

D = 512
H = 512
BFULL = 64
B = 16  # batch per core
CK = 4  # h chunks (H / 128)
MT = 16  # m tiles (4H / 128)
KT = 4  # d chunks (D / 128)
TFULL = 512


def build(T=TFULL, W=32, use_loop=True, loop_pairs=None, debug=False, finalize=True):
    """Build the per-core Bass program."""
    NW = T // W
    assert T % W == 0 and NW % 2 == 0
    NP = NW // 2  # window pairs
    if loop_pairs is None:
        loop_pairs = NP - 1 if use_loop else 0

    nc = bacc.Bacc(None, target_bir_lowering=False, debug=debug)

    xt_d = nc.dram_tensor("xt", [D, T, B], BF16, kind="ExternalInput")
    wih_d = nc.dram_tensor("wih", [D, 4 * H], BF16, kind="ExternalInput")
    whh_d = nc.dram_tensor("whh", [H, 4 * H], BF16, kind="ExternalInput")
    id_d = nc.dram_tensor("ident", [128, 128], BF16, kind="ExternalInput")
    out_d = nc.dram_tensor("out", [T, 128, 4 * B], BF16, kind="ExternalOutput")

    # out viewed per (pair, window-in-pair, step)
    out_v = out_d.rearrange("(np two w) p c -> np two w p c", two=2, w=W)

    with tile.TileContext(nc) as tc:
        from contextlib import ExitStack

        with ExitStack() as ctx:
            const = ctx.enter_context(tc.tile_pool(name="const", bufs=1))
            state = ctx.enter_context(tc.tile_pool(name="state", bufs=1))
            work = ctx.enter_context(tc.tile_pool(name="work", bufs=3))
            rec_ps = ctx.enter_context(tc.tile_pool(name="rec_ps", bufs=2, space="PSUM"))
            xg_ps = ctx.enter_context(tc.tile_pool(name="xg_ps", bufs=2, space="PSUM"))

            wih_sb = const.tile([128, KT, 4 * H], BF16, tag="wih")
            whh_sb = const.tile([128, CK, 4 * H], BF16, tag="whh")
            id_sb = const.tile([128, 128], BF16, tag="ident")

            hbf = [
                state.tile([128, CK * B], BF16, tag=f"hbf{j}", name=f"hbf{j}")
                for j in range(2)
            ]
            cst = [
                state.tile([128, CK * B], F32, tag=f"cst{j}", name=f"cst{j}")
                for j in range(2)
            ]
            xts = [
                state.tile([128, KT, W * B], BF16, tag=f"xt{j}", name=f"xtbuf{j}")
                for j in range(2)
            ]
            xgs = [
                state.tile([128, W * 256], BF16, tag=f"xg{j}", name=f"xgbuf{j}")
                for j in range(2)
            ]

            # ---- prologue ----
            for k in range(KT):
                nc.sync.dma_start(
                    out=wih_sb[:, k, :], in_=wih_d[k * 128 : (k + 1) * 128, :]
                )
                nc.sync.dma_start(
                    out=whh_sb[:, k, :], in_=whh_d[k * 128 : (k + 1) * 128, :]
                )
            nc.sync.dma_start(out=id_sb[:], in_=id_d[:])
            nc.vector.memset(hbf[0][:], 0.0)
            nc.vector.memset(cst[0][:], 0.0)

            def emit_xt_dma(win_expr, dst, k):
                dst_v = dst.rearrange("p k (s b) -> p k s b", b=B)
                nc.sync.dma_start(
                    out=dst_v[:, k],
                    in_=xt_d[k * 128 : (k + 1) * 128, ds(win_expr * W, W), :],
                )

            def emit_bulk_m(src_xt, dst_xg, m):
                # input-projection matmuls for one m-tile over a full window
                x_ps = xg_ps.tile([128, W * B], F32, tag="xps", name="xps")
                for k in range(KT):
                    nc.tensor.matmul(
                        x_ps[:],
                        wih_sb[:, k, m * 128 : (m + 1) * 128],
                        src_xt[:, k, :],
                        start=(k == 0),
                        stop=(k == KT - 1),
                    )
                src_v = x_ps.rearrange("p (s b) -> p s b", b=B)
                dst_v = dst_xg.rearrange("p (s x) -> p s x", x=256)[
                    :, :, m * 16 : (m + 1) * 16
                ]
                if m % 2 == 0:
                    nc.vector.tensor_copy(dst_v, src_v)
                else:
                    nc.scalar.copy(dst_v, src_v)

            def emit_step(wpair, wb, s, xg_sb):
                # one recurrent step; global t = (2*wpair+wb)*W + s
                par = s % 2
                h_prev, h_new = hbf[par], hbf[1 - par]
                c_prev, c_new = cst[par], cst[1 - par]

                g_ps = rec_ps.tile([128, 256], F32, tag="gps", name="gps")
                nc.tensor.matmul(
                    g_ps[:],
                    id_sb[:],
                    xg_sb[:, s * 256 : (s + 1) * 256],
                    start=True,
                    stop=False,
                )
                for m in range(MT):
                    out_sl = g_ps[:, m * 16 : (m + 1) * 16]
                    for k in range(CK):
                        nc.tensor.matmul(
                            out_sl,
                            whh_sb[:, k, m * 128 : (m + 1) * 128],
                            h_prev[:, k * B : (k + 1) * B],
                            start=False,
                            stop=(m == MT - 1 and k == CK - 1),
                        )

                g_v = g_ps.rearrange("p (c x) -> p c x", c=CK)  # [128,4,64]
                sif = work.tile([128, CK, 2 * B], F32, tag="sif", name="sif")
                tg = work.tile([128, CK, B], F32, tag="tg", name="tg")
                so = work.tile([128, CK, B], F32, tag="so", name="so")
                m1 = work.tile([128, CK, B], F32, tag="m1", name="m1")
                m2 = work.tile([128, CK, B], F32, tag="m2", name="m2")
                tch = work.tile([128, CK * B], F32, tag="tch", name="tch")

                nc.scalar.activation(sif[:], g_v[:, :, 0 : 2 * B], AF.Sigmoid)
                nc.scalar.activation(tg[:], g_v[:, :, 2 * B : 3 * B], AF.Tanh)
                nc.scalar.activation(so[:], g_v[:, :, 3 * B : 4 * B], AF.Sigmoid)

                c_prev_v = c_prev.rearrange("p (c b) -> p c b", b=B)
                c_new_v = c_new.rearrange("p (c b) -> p c b", b=B)
                nc.vector.tensor_mul(m1[:], sif[:, :, B : 2 * B], c_prev_v)
                nc.vector.tensor_mul(m2[:], sif[:, :, 0:B], tg[:])
                nc.vector.tensor_add(c_new_v, m1[:], m2[:])
                nc.scalar.activation(tch[:], c_new[:], AF.Tanh)
                tch_v = tch.rearrange("p (c b) -> p c b", b=B)
                h_new_v = h_new.rearrange("p (c b) -> p c b", b=B)
                nc.vector.tensor_mul(h_new_v, so[:], tch_v)

                if isinstance(wpair, int):
                    dst = out_v[wpair, wb, s]
                else:
                    dst = out_v[ds(wpair, 1), wb, s]
                nc.sync.dma_start(out=dst, in_=h_new[:])

            def emit_window(wpair, wb, xg_sb, tasks):
                n = len(tasks)
                done = 0
                for s in range(W):
                    emit_step(wpair, wb, s, xg_sb)
                    want = (s + 1) * n // W
                    while done < want:
                        tasks[done]()
                        done += 1

            def pair_tasks(i_expr, last):
                tA = []
                for m in range(MT):
                    tA.append(lambda m=m: emit_bulk_m(xts[1], xgs[1], m))
                tB = []
                if not last:
                    for k in range(KT):
                        tA.append(lambda k=k: emit_xt_dma(i_expr * 2 + 2, xts[0], k))
                    for m in range(MT):
                        tB.append(lambda m=m: emit_bulk_m(xts[0], xgs[0], m))
                    for k in range(KT):
                        tB.append(lambda k=k: emit_xt_dma(i_expr * 2 + 3, xts[1], k))
                return tA, tB

            # prologue: window 0 xg, window 0/1 xt
            for k in range(KT):
                emit_xt_dma(0, xts[0], k)
            for m in range(MT):
                emit_bulk_m(xts[0], xgs[0], m)
            if NW > 1:
                for k in range(KT):
                    emit_xt_dma(1, xts[1], k)

            def body(i_expr, last):
                tA, tB = pair_tasks(i_expr, last)
                emit_window(i_expr, 0, xgs[0], tA)
                emit_window(i_expr, 1, xgs[1], tB)

            if use_loop and loop_pairs > 0:
                with tc.For_i(0, loop_pairs) as iv:
                    body(iv, last=False)
                for p in range(loop_pairs, NP):
                    body(p, last=(p == NP - 1))
            else:
                for p in range(NP):
                    body(p, last=(p == NP - 1))

    if finalize:
        nc.finalize()
    else:
        nc.compile()
    return nc


# ---------------- host-side helpers ----------------

PERM = np.concatenate(
    [
        np.arange(g * H + c * 128, g * H + c * 128 + 128)
        for c in range(4)
        for g in range(4)
    ]
)


def pack_weights(Wih, Whh):
    bf = ml_dtypes.bfloat16
    wih_p = np.ascontiguousarray(np.asarray(Wih, np.float32)[PERM].T).astype(bf)
    whh_p = np.ascontiguousarray(np.asarray(Whh, np.float32)[PERM].T).astype(bf)
    ident = np.eye(128, dtype=bf)
    return wih_p, whh_p, ident


def pack_x(x_slice, reverse):
    # x_slice [B, T, D] float32 -> xt [D, T, B] bf16 (time-reversed for bw)
    bf = ml_dtypes.bfloat16
    xs = x_slice[:, ::-1, :] if reverse else x_slice
    return np.ascontiguousarray(xs.transpose(2, 1, 0)).astype(bf)


def unpack_out(out_dev, reverse):
    # out_dev [T, 128, 64] bf16 -> [T, H, B] float32
    T = out_dev.shape[0]
    o = out_dev.astype(np.float32).reshape(T, 128, 4, B)
    o = o.transpose(0, 2, 1, 3).reshape(T, H, B)
    if reverse:
        o = o[::-1]
    return o


_NC_CACHE = {}


def _get_nc():
    key = "default"
    if key not in _NC_CACHE:
        _NC_CACHE[key] = build()
    return _NC_CACHE[key]


def run(x, Wih_fw, Whh_fw, Wih_bw, Whh_bw, trace=False, tmpdir=None):
    x = np.asarray(x, np.float32)
    wf = pack_weights(Wih_fw, Whh_fw)
    wb = pack_weights(Wih_bw, Whh_bw)
    in_maps = []
    for core in range(8):
        rev = core >= 4
        sl = core % 4
        wih_p, whh_p, ident = wb if rev else wf
        in_maps.append(
            {
                "xt": pack_x(x[sl * B : (sl + 1) * B], rev),
                "wih": wih_p,
                "whh": whh_p,
                "ident": ident,
            }
        )
    kw = {}
    if trace:
        kw["trace"] = True
        if tmpdir is not None:
            kw["tmpdir"] = tmpdir
    res = run_bass_kernel_spmd(_get_nc(), in_maps, core_ids=list(range(8)), **kw)
    out = np.zeros((TFULL, BFULL, H), np.float32)
    for sl in range(4):
        fw = unpack_out(np.asarray(res.results[sl]["out"]), False)
        bw = unpack_out(np.asarray(res.results[4 + sl]["out"]), True)
        out[:, sl * B : (sl + 1) * B, :] = (fw + bw).transpose(0, 2, 1)
    return out, res


def kernel(x, Wih_fw, Whh_fw, Wih_bw, Whh_bw):
    out, _ = run(x, Wih_fw, Whh_fw, Wih_bw, Whh_bw)
    return out
